# revision 17
# baseline (speedup 1.0000x reference)
"""CvT attention block on 8 trn2 NeuronCores — batch-parallel (1 image/core).

v7: bf16 inputs xbar-transposed to staging then copied into padded planes;
weight DMAs on the second HWDGE queue (scalar) in parallel; head-pair QK
psum + single N=896 exp ACTIVATE per kv-chunk; softmax normalize decoupled
from PSUM via early SBUF copy; next-tile q-path interleaved into attention;
pw bias on vector; bf16 out-projection.
"""
import sys

if '/opt/trn_rl_repo' not in sys.path:
    sys.path.insert(0, '/opt/trn_rl_repo')

from contextlib import ExitStack

import numpy as np
import ml_dtypes

import concourse.bass as bass
import concourse.tile as tile
from concourse import mybir, bacc
from concourse.bass_utils import run_bass_kernel_spmd

F32 = mybir.dt.float32
BF16 = mybir.dt.bfloat16
AF = mybir.ActivationFunctionType
ALU = mybir.AluOpType

B, H, W, C = 8, 56, 56, 384
NH, HD = 6, 64
NTOK = H * W            # 3136
NKV = 28 * 28           # 784
PW = 58                 # padded plane width
QT = 448                # q token tile = 8 image rows
NQT = NTOK // QT        # 7
KVC = 112               # kv chunk (attention contraction tile)
NKVC = NKV // KVC       # 7
BN_EPS = 1e-5

_cache = {}


def _build_nc():
    nc = bacc.Bacc("TRN2", target_bir_lowering=False, debug=False)
    d = {}
    ext = lambda n, s, dt: nc.dram_tensor(n, s, dt, kind="ExternalInput").ap()
    d['xq'] = ext("xq", [NTOK, C], BF16)
    d['xkv'] = ext("xkv", [NTOK, C], BF16)
    for p in 'qkv':
        d[f'w{p}'] = ext(f"w{p}", [C, C], BF16)                 # pw weights (cin, cout)
        d[f'diag{p}'] = ext(f"diag{p}", [128, 27 * 128], BF16)  # diag blocks (tap*3+ch)
    d['bq'] = ext("bq", [128, 3], F32)
    d['bk'] = ext("bk", [128, 3], F32)
    d['bv'] = ext("bv", [1, C], BF16)
    d['ok'] = ext("ok", [C, C], BF16)                           # out_kernel (hd, o)
    out_d = nc.dram_tensor("out", [NTOK, C], F32, kind="ExternalOutput").ap()

    with nc.allow_low_precision(reason="bf16 rounding is intentional"), \
         tile.TileContext(nc) as tc, ExitStack() as ctx:
        wp = ctx.enter_context(tc.tile_pool(name="wp", bufs=1))
        padp = ctx.enter_context(tc.tile_pool(name="padp", bufs=1))
        dwo = ctx.enter_context(tc.tile_pool(name="dwo", bufs=1))
        dwq = ctx.enter_context(tc.tile_pool(name="dwq", bufs=2))
        actp = ctx.enter_context(tc.tile_pool(name="actp", bufs=1))
        qtp = ctx.enter_context(tc.tile_pool(name="qtp", bufs=2))
        ptp = ctx.enter_context(tc.tile_pool(name="ptp", bufs=1))
        smp = ctx.enter_context(tc.tile_pool(name="smp", bufs=2))
        ap_ = ctx.enter_context(tc.tile_pool(name="ap", bufs=2))
        outp = ctx.enter_context(tc.tile_pool(name="outp", bufs=2))
        psu = ctx.enter_context(tc.tile_pool(name="psu", bufs=1, space="PSUM"))
        _n = [0]

        def nm(s):
            _n[0] += 1
            return f"{s}_{_n[0]}"

        def load(name, shape, tag, dt, rows=None):
            t = wp.tile(shape, dt, tag=tag, name=nm(tag))
            src = d[name][:, :] if rows is None else d[name][rows[0]:rows[1], :]
            nc.scalar.dma_start(t[:], src)
            return t

        pads_kv = [padp.tile([128, PW * PW], BF16, tag=f"pad{ch}", name=nm(f"pad{ch}"))
                   for ch in range(3)]
        pads_q = [padp.tile([128, PW * PW], BF16, tag=f"padq{ch}", name=nm(f"padq{ch}"))
                  for ch in range(3)]
        zrow = wp.tile([128, PW], BF16, tag="zrow", name=nm("zrow"))
        nc.vector.memset(zrow[:], 0.0)

        def zero_border(pad):
            pv = pad[:].rearrange("p (r c) -> p r c", c=PW)
            zr = zrow[:].rearrange("p (a c) -> p a c", a=1)
            zc = zrow[:].rearrange("p (c a) -> p c a", a=1)
            nc.vector.tensor_copy(pv[:, 0:1, :], zr)
            nc.vector.tensor_copy(pv[:, PW - 1:PW, :], zr)
            nc.vector.tensor_copy(pv[:, :, 0:1], zc)
            nc.vector.tensor_copy(pv[:, :, PW - 1:PW], zc)

        for ch in range(3):
            zero_border(pads_kv[ch])
            zero_border(pads_q[ch])

        xstg = ctx.enter_context(tc.tile_pool(name="xstg", bufs=3))

        def load_plane(src_dram, pads, ch, on_scalar):
            """xbar-transpose token-major [3136, 128] to channel-major staging,
            then one strided copy into the zero-padded plane interior."""
            st = xstg.tile([128, NTOK], BF16, tag="xstg", name=nm("xstg"))
            nc.sync.dma_start_transpose(st[:], src_dram[:, ch * 128:(ch + 1) * 128])
            pv = pads[ch][:].rearrange("p (r c) -> p r c", c=PW)
            sv = st[:].rearrange("p (r c) -> p r c", c=56)
            if on_scalar:
                nc.scalar.copy(pv[:, 1:57, 1:57], sv)
            else:
                nc.vector.tensor_copy(pv[:, 1:57, 1:57], sv)

        diags = {}

        def load_diag(p):
            t = wp.tile([128, 27 * 128], BF16, tag=f"diag{p}", name=nm(f"diag{p}"))
            nc.scalar.dma_start(t[:], d[f'diag{p}'][:, :])
            diags[p] = t

        def dg(p, ch, tap):
            b = tap * 3 + ch
            return diags[p][:, b * 128:(b + 1) * 128]

        kvdw = {(p, ch): dwo.tile([128, NKV], BF16, tag=f"kvdw_{p}{ch}",
                                  name=nm(f"kvdw_{p}{ch}"))
                for p in 'kv' for ch in range(3)}

        def dw_kv(p, ch):
            pv = pads_kv[ch][:].rearrange("p (r c) -> p r c", c=PW)
            for half in range(2):
                ps = psu.tile([128, 392], F32, tag="pb", name=nm("pb"), bufs=2)
                for tap in range(9):
                    dy, dx = tap // 3, tap % 3
                    y0 = half * 14
                    rv = pv[:, 2 * y0 + dy + 1: 2 * y0 + dy + 28:2,
                            dx + 1: dx + 56:2]
                    nc.tensor.matmul(ps[:], dg(p, ch, tap), rv,
                                     start=(tap == 0), stop=(tap == 8))
                dst = kvdw[(p, ch)][:, half * 392:(half + 1) * 392]
                if (ch + half) % 2 == 0:
                    nc.vector.tensor_copy(dst, ps[:])
                else:
                    nc.scalar.copy(dst, ps[:])

        # ---- setup: weights on scalar queue, planes back-to-back on sync ----
        ones1 = wp.tile([1, KVC], F32, tag="ones1", name=nm("ones1"))
        nc.vector.memset(ones1[:], 1.0)
        ones1r = wp.tile([1, KVC], BF16, tag="ones1r", name=nm("ones1r"))
        nc.vector.tensor_copy(ones1r[:], ones1[:])
        onesv = wp.tile([112, NH], F32, tag="onesv", name=nm("onesv"))
        nc.vector.memset(onesv[:], 1.0)
        load_diag('k')
        load_diag('v')
        load_diag('q')
        wmat = {}
        wmat['k'] = [load('wk', [128, C], f"wk{c}", BF16, rows=(c * 128, (c + 1) * 128))
                     for c in range(3)]
        bq = load('bq', [128, 3], "bq", F32)
        bk = load('bk', [128, 3], "bk", F32)
        bv = load('bv', [1, C], "bv", BF16)
        wmat['v'] = [load('wv', [128, C], f"wv{c}", BF16, rows=(c * 128, (c + 1) * 128))
                     for c in range(3)]
        wmat['q'] = [load('wq', [128, C], f"wq{c}", BF16, rows=(c * 128, (c + 1) * 128))
                     for c in range(3)]
        okm = [load('ok', [128, C], f"ok{c}", BF16, rows=(c * 128, (c + 1) * 128))
               for c in range(3)]
        load_plane(d['xkv'], pads_kv, 0, on_scalar=False)
        load_plane(d['xkv'], pads_kv, 1, on_scalar=True)
        load_plane(d['xkv'], pads_kv, 2, on_scalar=False)
        dw_kv('k', 0)
        dw_kv('v', 0)
        load_plane(d['xq'], pads_q, 0, on_scalar=True)
        dw_kv('k', 1)
        dw_kv('v', 1)
        load_plane(d['xq'], pads_q, 1, on_scalar=False)
        dw_kv('k', 2)
        dw_kv('v', 2)
        load_plane(d['xq'], pads_q, 2, on_scalar=True)

        # pw-k: channel-major kT [3][128, 784]
        kT = []
        for co in range(3):
            kt = actp.tile([128, NKV], BF16, tag=f"kT{co}", name=nm(f"kT{co}"))
            kT.append(kt)
            for half in range(2):
                ps = psu.tile([128, 392], F32, tag="pb", name=nm("pb"), bufs=2)
                for ci in range(3):
                    nc.tensor.matmul(
                        ps[:], wmat['k'][ci][:, co * 128:(co + 1) * 128],
                        kvdw[('k', ci)][:, half * 392:(half + 1) * 392],
                        start=(ci == 0), stop=(ci == 2))
                nc.scalar.activation(kt[:, half * 392:(half + 1) * 392], ps[:],
                                     AF.Identity, bias=bk[:, co:co + 1])

        # pw-v: token-major v' [7][112, 6*65] with ones col per head
        vs = []
        for j in range(NKVC):
            ps = psu.tile([112, C], F32, tag="pb", name=nm("pb"), bufs=2)
            for ci in range(3):
                nc.tensor.matmul(ps[:], kvdw[('v', ci)][:, j * KVC:(j + 1) * KVC],
                                 wmat['v'][ci][:], start=(ci == 0), stop=False)
            nc.tensor.matmul(ps[:], ones1r[:], bv[:], start=False, stop=True)
            vt = actp.tile([112, NH * 65], BF16, tag=f"vs{j}", name=nm(f"vs{j}"))
            vs.append(vt)
            vv = vt[:].rearrange("p (h e) -> p h e", e=65)
            nc.vector.tensor_copy(vv[:, :, 0:64],
                                  ps[:].rearrange("p (h e) -> p h e", e=64))
            nc.vector.tensor_copy(vv[:, :, 64:65],
                                  onesv[:].rearrange("p (h e) -> p h e", e=1))

        # ---- Q path + attention + projection, per 448-token tile ----
        def dwq_ch(t, ch):
            y0 = t * 8
            ps = psu.tile([128, QT], F32, tag="pb", name=nm("pb"), bufs=2)
            pv = pads_q[ch][:].rearrange("p (r c) -> p r c", c=PW)
            for tap in range(9):
                dy, dx = tap // 3, tap % 3
                rv = pv[:, y0 + dy:y0 + dy + 8, dx:dx + 56]
                nc.tensor.matmul(ps[:], dg('q', ch, tap), rv,
                                 start=(tap == 0), stop=(tap == 8))
            dt_ = dwq.tile([128, QT], BF16, tag=f"dwq{ch}", name=nm(f"dwq{ch}"), bufs=3)
            nc.vector.tensor_copy(dt_[:], ps[:])
            return dt_

        def pwq_co(dq, co):
            ps = psu.tile([128, QT], F32, tag="pb", name=nm("pb"), bufs=2)
            for ci in range(3):
                nc.tensor.matmul(ps[:], wmat['q'][ci][:, co * 128:(co + 1) * 128],
                                 dq[ci][:], start=(ci == 0), stop=(ci == 2))
            qtt = qtp.tile([128, QT], BF16, tag=f"qt{co}", name=nm(f"qt{co}"), bufs=2)
            nc.vector.tensor_scalar(qtt[:], ps[:], bq[:, co:co + 1], None,
                                    op0=ALU.add)
            return qtt

        # prologue: q-path for tile 0
        dq0 = [dwq_ch(0, ch) for ch in range(3)]
        qt_cur = [pwq_co(dq0, co) for co in range(3)]

        for t in range(NQT):
            qt_ = qt_cur
            dq_next = []

            # attention: per ch, head pair (partitions 0-63 / 64-127) row-tiled
            at_ = [ap_.tile([128, QT], BF16, tag=f"at{ch}", name=nm(f"at{ch}"))
                   for ch in range(3)]
            for ch in range(3):
                mp0 = psu.tile([65, QT], F32, tag="mp", name=nm("mp"), bufs=2)
                mp1 = psu.tile([65, QT], F32, tag="mp", name=nm("mp"), bufs=2)
                mps = (mp0, mp1)
                for j in range(NKVC):
                    ps = psu.tile([112, 1024], F32, tag="qk", name=nm("qk"), bufs=2)
                    for half in range(2):
                        lo = half * 64
                        nc.tensor.matmul(ps[:, half * 512:half * 512 + QT],
                                         kT[ch][lo:lo + 64, j * KVC:(j + 1) * KVC],
                                         qt_[ch][lo:lo + 64, :], start=True, stop=True,
                                         skip_group_check=True)
                    pt = ptp.tile([112, 2 * QT], BF16, tag=f"ptj{j}",
                                  name=nm(f"ptj{j}"), bufs=2)
                    nc.scalar.activation(
                        pt[:].rearrange("p (a q) -> p a q", a=2),
                        ps[:].rearrange("p (a q) -> p a q", a=2)[:, :, 0:QT],
                        AF.Exp)
                    for half in range(2):
                        h = 2 * ch + half
                        nc.tensor.matmul(mps[half][:], vs[j][:, h * 65:(h + 1) * 65],
                                         pt[:, half * QT:(half + 1) * QT],
                                         start=(j == 0), stop=(j == NKVC - 1))
                for half in range(2):
                    av = mps[half]
                    den = smp.tile([1, QT], F32, tag="den", name=nm("den"))
                    nc.vector.tensor_copy(den[:], av[64:65, :])
                    rec = smp.tile([1, QT], F32, tag="rec", name=nm("rec"))
                    nc.vector.reciprocal_approx_fast(rec[:], den[:])
                    bc = smp.tile([64, QT], F32, tag="bc", name=nm("bc"), bufs=2)
                    nc.gpsimd.partition_broadcast(bc[:], rec[:])
                    nc.vector.tensor_tensor(at_[ch][half * 64:half * 64 + 64, :],
                                            av[0:64, :], bc[:],
                                            op=ALU.mult)
                if t + 1 < NQT:
                    dq_next.append(dwq_ch(t + 1, ch))

            if t + 1 < NQT:
                qt_cur = [pwq_co(dq_next, co) for co in range(3)]

            # out projection, token-major
            for i, qn in ((0, 128), (1, 128), (2, 128), (3, 64)):
                ps = psu.tile([qn, C], F32, tag="mp", name=nm("mp"), bufs=2)
                for ch in range(3):
                    nc.tensor.matmul(ps[:], at_[ch][:, i * 128:i * 128 + qn],
                                     okm[ch][:], start=(ch == 0), stop=(ch == 2))
                ot = outp.tile([qn, C], F32, tag="ot", name=nm("ot"))
                nc.vector.tensor_copy(ot[:], ps[:])
                nc.sync.dma_start(out_d[t * QT + i * 128:t * QT + i * 128 + qn, :],
                                  ot[:])

    nc.compile()
    return nc


def _fold_weights(inputs):
    g = lambda n: np.asarray(inputs[n], dtype=np.float32)
    bf = lambda a: np.ascontiguousarray(a).astype(ml_dtypes.bfloat16)
    fold = {}
    for p in 'qkv':
        s = g(f'{p}_bn_scale') / np.sqrt(g(f'{p}_bn_var') + BN_EPS)
        t = g(f'{p}_bn_bias') - g(f'{p}_bn_mean') * s
        dw = g(f'{p}_dw_kernel').reshape(9, C) * s[None, :]         # (tap, c)
        wmat = g(f'{p}_pw_kernel').reshape(C, C)
        bias = t @ wmat
        if p == 'q':
            wmat = wmat / np.sqrt(np.float32(HD))
            bias = bias / np.sqrt(np.float32(HD))
        diag = np.zeros((128, 27 * 128), dtype=np.float32)
        for tap in range(9):
            for ch in range(3):
                b = tap * 3 + ch
                diag[:, b * 128:(b + 1) * 128] = np.diag(dw[tap, ch * 128:(ch + 1) * 128])
        fold[f'w{p}'] = bf(wmat)
        fold[f'diag{p}'] = bf(diag)
        fold[f'b{p}'] = bias
    common = {
        'wq': fold['wq'], 'wk': fold['wk'], 'wv': fold['wv'],
        'diagq': fold['diagq'], 'diagk': fold['diagk'], 'diagv': fold['diagv'],
        'bq': np.ascontiguousarray(fold['bq'].reshape(3, 128).T),
        'bk': np.ascontiguousarray(fold['bk'].reshape(3, 128).T),
        'bv': bf(fold['bv'].reshape(1, C)),
        'ok': bf(np.asarray(inputs['out_kernel'], dtype=np.float32).reshape(C, C)),
    }
    return common


def kernel(**inputs):
    if 'nc' not in _cache:
        _cache['nc'] = _build_nc()
    nc = _cache['nc']
    common = _fold_weights(inputs)
    bf = ml_dtypes.bfloat16
    xq = np.asarray(inputs['inputs_q'], dtype=np.float32).reshape(B, NTOK, C).astype(bf)
    xkv = np.asarray(inputs['inputs_kv'], dtype=np.float32).reshape(B, NTOK, C).astype(bf)
    in_maps = [dict(common, xq=np.ascontiguousarray(xq[b]),
                    xkv=np.ascontiguousarray(xkv[b])) for b in range(B)]
    res = run_bass_kernel_spmd(nc, in_maps, list(range(B)), trace=False)
    out = np.stack([res.results[b]['out'] for b in range(B)], axis=0)
    return out.astype(np.float32)


# revision 18
# speedup vs baseline: 1.1026x; 1.1026x over previous
"""CvT attention block on 8 trn2 NeuronCores — batch-parallel (1 image/core).

v7: bf16 inputs xbar-transposed to staging then copied into padded planes;
weight DMAs on the second HWDGE queue (scalar) in parallel; head-pair QK
psum + single N=896 exp ACTIVATE per kv-chunk; softmax normalize decoupled
from PSUM via early SBUF copy; next-tile q-path interleaved into attention;
pw bias on vector; bf16 out-projection.
"""
import sys

if '/opt/trn_rl_repo' not in sys.path:
    sys.path.insert(0, '/opt/trn_rl_repo')

from contextlib import ExitStack

import numpy as np
import ml_dtypes

import concourse.bass as bass
import concourse.tile as tile
from concourse import mybir, bacc
from concourse.bass_utils import run_bass_kernel_spmd

F32 = mybir.dt.float32
BF16 = mybir.dt.bfloat16
AF = mybir.ActivationFunctionType
ALU = mybir.AluOpType

B, H, W, C = 8, 56, 56, 384
NH, HD = 6, 64
NTOK = H * W            # 3136
NKV = 28 * 28           # 784
PW = 58                 # padded plane width
QT = 448                # q token tile = 8 image rows
NQT = NTOK // QT        # 7
KVC = 112               # kv chunk (attention contraction tile)
NKVC = NKV // KVC       # 7
BN_EPS = 1e-5

_cache = {}


def _build_nc():
    nc = bacc.Bacc("TRN2", target_bir_lowering=False, debug=False)
    d = {}
    ext = lambda n, s, dt: nc.dram_tensor(n, s, dt, kind="ExternalInput").ap()
    d['xq'] = ext("xq", [NTOK, C], BF16)
    d['xkv'] = ext("xkv", [NTOK, C], BF16)
    for p in 'qkv':
        d[f'w{p}'] = ext(f"w{p}", [C, C], BF16)                 # pw weights (cin, cout)
        d[f'diag{p}'] = ext(f"diag{p}", [128, 27 * 128], BF16)  # diag blocks (tap*3+ch)
    d['bq'] = ext("bq", [128, 3], F32)
    d['bk'] = ext("bk", [128, 3], F32)
    d['bv'] = ext("bv", [1, C], BF16)
    d['ok'] = ext("ok", [C, C], BF16)                           # out_kernel (hd, o)
    out_d = nc.dram_tensor("out", [NTOK, C], F32, kind="ExternalOutput").ap()

    with nc.allow_low_precision(reason="bf16 rounding is intentional"), \
         tile.TileContext(nc) as tc, ExitStack() as ctx:
        wp = ctx.enter_context(tc.tile_pool(name="wp", bufs=1))
        padp = ctx.enter_context(tc.tile_pool(name="padp", bufs=1))
        dwo = ctx.enter_context(tc.tile_pool(name="dwo", bufs=1))
        dwq = ctx.enter_context(tc.tile_pool(name="dwq", bufs=2))
        actp = ctx.enter_context(tc.tile_pool(name="actp", bufs=1))
        qtp = ctx.enter_context(tc.tile_pool(name="qtp", bufs=2))
        ptp = ctx.enter_context(tc.tile_pool(name="ptp", bufs=1))
        smp = ctx.enter_context(tc.tile_pool(name="smp", bufs=2))
        ap_ = ctx.enter_context(tc.tile_pool(name="ap", bufs=2))
        outp = ctx.enter_context(tc.tile_pool(name="outp", bufs=2))
        psu = ctx.enter_context(tc.tile_pool(name="psu", bufs=1, space="PSUM"))
        _n = [0]

        def nm(s):
            _n[0] += 1
            return f"{s}_{_n[0]}"

        def load(name, shape, tag, dt, rows=None):
            t = wp.tile(shape, dt, tag=tag, name=nm(tag))
            src = d[name][:, :] if rows is None else d[name][rows[0]:rows[1], :]
            nc.sync.dma_start(t[:], src)
            return t

        pads_kv = [padp.tile([128, PW * PW], BF16, tag=f"pad{ch}", name=nm(f"pad{ch}"))
                   for ch in range(3)]
        pads_q = [padp.tile([128, PW * PW], BF16, tag=f"padq{ch}", name=nm(f"padq{ch}"))
                  for ch in range(3)]
        zrow = wp.tile([128, PW], BF16, tag="zrow", name=nm("zrow"))
        nc.vector.memset(zrow[:], 0.0)

        def zero_border(pad):
            pv = pad[:].rearrange("p (r c) -> p r c", c=PW)
            zr = zrow[:].rearrange("p (a c) -> p a c", a=1)
            zc = zrow[:].rearrange("p (c a) -> p c a", a=1)
            nc.vector.tensor_copy(pv[:, 0:1, :], zr)
            nc.vector.tensor_copy(pv[:, PW - 1:PW, :], zr)
            nc.vector.tensor_copy(pv[:, :, 0:1], zc)
            nc.vector.tensor_copy(pv[:, :, PW - 1:PW], zc)

        for ch in range(3):
            zero_border(pads_kv[ch])
            zero_border(pads_q[ch])

        xstg = ctx.enter_context(tc.tile_pool(name="xstg", bufs=3))

        def load_plane(src_dram, pads, ch, on_scalar):
            """xbar-transpose token-major [3136, 128] to channel-major staging,
            then one strided copy into the zero-padded plane interior."""
            st = xstg.tile([128, NTOK], BF16, tag="xstg", name=nm("xstg"))
            nc.sync.dma_start_transpose(st[:], src_dram[:, ch * 128:(ch + 1) * 128])
            pv = pads[ch][:].rearrange("p (r c) -> p r c", c=PW)
            sv = st[:].rearrange("p (r c) -> p r c", c=56)
            if on_scalar:
                nc.scalar.copy(pv[:, 1:57, 1:57], sv)
            else:
                nc.vector.tensor_copy(pv[:, 1:57, 1:57], sv)

        diags = {}

        def load_diag(p):
            t = wp.tile([128, 27 * 128], BF16, tag=f"diag{p}", name=nm(f"diag{p}"))
            nc.sync.dma_start(t[:], d[f'diag{p}'][:, :])
            diags[p] = t

        def dg(p, ch, tap):
            b = tap * 3 + ch
            return diags[p][:, b * 128:(b + 1) * 128]

        kvdw = {(p, ch): dwo.tile([128, NKV], BF16, tag=f"kvdw_{p}{ch}",
                                  name=nm(f"kvdw_{p}{ch}"))
                for p in 'kv' for ch in range(3)}

        def dw_kv(p, ch):
            pv = pads_kv[ch][:].rearrange("p (r c) -> p r c", c=PW)
            for half in range(2):
                ps = psu.tile([128, 392], F32, tag="pb", name=nm("pb"), bufs=2)
                for tap in range(9):
                    dy, dx = tap // 3, tap % 3
                    y0 = half * 14
                    rv = pv[:, 2 * y0 + dy + 1: 2 * y0 + dy + 28:2,
                            dx + 1: dx + 56:2]
                    nc.tensor.matmul(ps[:], dg(p, ch, tap), rv,
                                     start=(tap == 0), stop=(tap == 8))
                dst = kvdw[(p, ch)][:, half * 392:(half + 1) * 392]
                if (ch + half) % 2 == 0:
                    nc.vector.tensor_copy(dst, ps[:])
                else:
                    nc.scalar.copy(dst, ps[:])

        # ---- setup: weights on scalar queue, planes back-to-back on sync ----
        ones1 = wp.tile([1, KVC], F32, tag="ones1", name=nm("ones1"))
        nc.vector.memset(ones1[:], 1.0)
        ones1r = wp.tile([1, KVC], BF16, tag="ones1r", name=nm("ones1r"))
        nc.vector.tensor_copy(ones1r[:], ones1[:])
        onesv = wp.tile([112, NH], F32, tag="onesv", name=nm("onesv"))
        nc.vector.memset(onesv[:], 1.0)
        load_diag('k')
        load_diag('v')
        load_diag('q')
        wmat = {}
        wmat['k'] = [load('wk', [128, C], f"wk{c}", BF16, rows=(c * 128, (c + 1) * 128))
                     for c in range(3)]
        bq = load('bq', [128, 3], "bq", F32)
        bk = load('bk', [128, 3], "bk", F32)
        bv = load('bv', [1, C], "bv", BF16)
        wmat['v'] = [load('wv', [128, C], f"wv{c}", BF16, rows=(c * 128, (c + 1) * 128))
                     for c in range(3)]
        wmat['q'] = [load('wq', [128, C], f"wq{c}", BF16, rows=(c * 128, (c + 1) * 128))
                     for c in range(3)]
        okm = [load('ok', [128, C], f"ok{c}", BF16, rows=(c * 128, (c + 1) * 128))
               for c in range(3)]
        load_plane(d['xkv'], pads_kv, 0, on_scalar=False)
        load_plane(d['xkv'], pads_kv, 1, on_scalar=True)
        load_plane(d['xkv'], pads_kv, 2, on_scalar=False)
        dw_kv('k', 0)
        dw_kv('v', 0)
        load_plane(d['xq'], pads_q, 0, on_scalar=True)
        dw_kv('k', 1)
        dw_kv('v', 1)
        load_plane(d['xq'], pads_q, 1, on_scalar=False)
        dw_kv('k', 2)
        dw_kv('v', 2)
        load_plane(d['xq'], pads_q, 2, on_scalar=True)

        # pw-k: channel-major kT [3][128, 784]
        kT = []
        for co in range(3):
            kt = actp.tile([128, NKV], BF16, tag=f"kT{co}", name=nm(f"kT{co}"))
            kT.append(kt)
            for half in range(2):
                ps = psu.tile([128, 392], F32, tag="pb", name=nm("pb"), bufs=2)
                for ci in range(3):
                    nc.tensor.matmul(
                        ps[:], wmat['k'][ci][:, co * 128:(co + 1) * 128],
                        kvdw[('k', ci)][:, half * 392:(half + 1) * 392],
                        start=(ci == 0), stop=(ci == 2))
                nc.scalar.activation(kt[:, half * 392:(half + 1) * 392], ps[:],
                                     AF.Identity, bias=bk[:, co:co + 1])

        # pw-v: token-major v' [7][112, 6*65] with ones col per head
        vs = []
        for j in range(NKVC):
            ps = psu.tile([112, C], F32, tag="pb", name=nm("pb"), bufs=2)
            for ci in range(3):
                nc.tensor.matmul(ps[:], kvdw[('v', ci)][:, j * KVC:(j + 1) * KVC],
                                 wmat['v'][ci][:], start=(ci == 0), stop=False)
            nc.tensor.matmul(ps[:], ones1r[:], bv[:], start=False, stop=True)
            vt = actp.tile([112, NH * 65], BF16, tag=f"vs{j}", name=nm(f"vs{j}"))
            vs.append(vt)
            vv = vt[:].rearrange("p (h e) -> p h e", e=65)
            nc.vector.tensor_copy(vv[:, :, 0:64],
                                  ps[:].rearrange("p (h e) -> p h e", e=64))
            nc.vector.tensor_copy(vv[:, :, 64:65],
                                  onesv[:].rearrange("p (h e) -> p h e", e=1))

        # ---- Q path + attention + projection, per 448-token tile ----
        def dwq_ch(t, ch):
            y0 = t * 8
            ps = psu.tile([128, QT], F32, tag="pb", name=nm("pb"), bufs=2)
            pv = pads_q[ch][:].rearrange("p (r c) -> p r c", c=PW)
            for tap in range(9):
                dy, dx = tap // 3, tap % 3
                rv = pv[:, y0 + dy:y0 + dy + 8, dx:dx + 56]
                nc.tensor.matmul(ps[:], dg('q', ch, tap), rv,
                                 start=(tap == 0), stop=(tap == 8))
            dt_ = dwq.tile([128, QT], BF16, tag=f"dwq{ch}", name=nm(f"dwq{ch}"), bufs=3)
            nc.vector.tensor_copy(dt_[:], ps[:])
            return dt_

        def pwq_co(dq, co):
            ps = psu.tile([128, QT], F32, tag="pb", name=nm("pb"), bufs=2)
            for ci in range(3):
                nc.tensor.matmul(ps[:], wmat['q'][ci][:, co * 128:(co + 1) * 128],
                                 dq[ci][:], start=(ci == 0), stop=(ci == 2))
            qtt = qtp.tile([128, QT], BF16, tag=f"qt{co}", name=nm(f"qt{co}"), bufs=2)
            nc.vector.tensor_scalar(qtt[:], ps[:], bq[:, co:co + 1], None,
                                    op0=ALU.add)
            return qtt

        # prologue: q-path for tile 0
        dq0 = [dwq_ch(0, ch) for ch in range(3)]
        qt_cur = [pwq_co(dq0, co) for co in range(3)]

        for t in range(NQT):
            qt_ = qt_cur
            dq_next = []

            # attention: per ch, head pair (partitions 0-63 / 64-127) row-tiled
            at_ = [ap_.tile([128, QT], BF16, tag=f"at{ch}", name=nm(f"at{ch}"))
                   for ch in range(3)]
            for ch in range(3):
                mp0 = psu.tile([65, QT], F32, tag="mp", name=nm("mp"), bufs=2)
                mp1 = psu.tile([65, QT], F32, tag="mp", name=nm("mp"), bufs=2)
                mps = (mp0, mp1)
                for j in range(NKVC):
                    ps = psu.tile([112, 1024], F32, tag="qk", name=nm("qk"), bufs=2)
                    for half in range(2):
                        lo = half * 64
                        nc.tensor.matmul(ps[:, half * 512:half * 512 + QT],
                                         kT[ch][lo:lo + 64, j * KVC:(j + 1) * KVC],
                                         qt_[ch][lo:lo + 64, :], start=True, stop=True,
                                         skip_group_check=True)
                    pt = ptp.tile([112, 2 * QT], BF16, tag=f"ptj{j}",
                                  name=nm(f"ptj{j}"), bufs=2)
                    nc.scalar.activation(
                        pt[:].rearrange("p (a q) -> p a q", a=2),
                        ps[:].rearrange("p (a q) -> p a q", a=2)[:, :, 0:QT],
                        AF.Exp)
                    for half in range(2):
                        h = 2 * ch + half
                        nc.tensor.matmul(mps[half][:], vs[j][:, h * 65:(h + 1) * 65],
                                         pt[:, half * QT:(half + 1) * QT],
                                         start=(j == 0), stop=(j == NKVC - 1))
                for half in range(2):
                    av = mps[half]
                    den = smp.tile([1, QT], F32, tag="den", name=nm("den"))
                    nc.vector.tensor_copy(den[:], av[64:65, :])
                    rec = smp.tile([1, QT], F32, tag="rec", name=nm("rec"))
                    nc.vector.reciprocal_approx_fast(rec[:], den[:])
                    bc = smp.tile([64, QT], F32, tag="bc", name=nm("bc"), bufs=2)
                    nc.gpsimd.partition_broadcast(bc[:], rec[:])
                    nc.vector.tensor_tensor(at_[ch][half * 64:half * 64 + 64, :],
                                            av[0:64, :], bc[:],
                                            op=ALU.mult)
                if t + 1 < NQT:
                    dq_next.append(dwq_ch(t + 1, ch))

            if t + 1 < NQT:
                qt_cur = [pwq_co(dq_next, co) for co in range(3)]

            # out projection, token-major
            for i, qn in ((0, 128), (1, 128), (2, 128), (3, 64)):
                ps = psu.tile([qn, C], F32, tag="mp", name=nm("mp"), bufs=2)
                for ch in range(3):
                    nc.tensor.matmul(ps[:], at_[ch][:, i * 128:i * 128 + qn],
                                     okm[ch][:], start=(ch == 0), stop=(ch == 2))
                ot = outp.tile([qn, C], F32, tag="ot", name=nm("ot"))
                nc.vector.tensor_copy(ot[:], ps[:])
                nc.sync.dma_start(out_d[t * QT + i * 128:t * QT + i * 128 + qn, :],
                                  ot[:])

    nc.compile()
    return nc


def _fold_weights(inputs):
    g = lambda n: np.asarray(inputs[n], dtype=np.float32)
    bf = lambda a: np.ascontiguousarray(a).astype(ml_dtypes.bfloat16)
    fold = {}
    for p in 'qkv':
        s = g(f'{p}_bn_scale') / np.sqrt(g(f'{p}_bn_var') + BN_EPS)
        t = g(f'{p}_bn_bias') - g(f'{p}_bn_mean') * s
        dw = g(f'{p}_dw_kernel').reshape(9, C) * s[None, :]         # (tap, c)
        wmat = g(f'{p}_pw_kernel').reshape(C, C)
        bias = t @ wmat
        if p == 'q':
            wmat = wmat / np.sqrt(np.float32(HD))
            bias = bias / np.sqrt(np.float32(HD))
        diag = np.zeros((128, 27 * 128), dtype=np.float32)
        for tap in range(9):
            for ch in range(3):
                b = tap * 3 + ch
                diag[:, b * 128:(b + 1) * 128] = np.diag(dw[tap, ch * 128:(ch + 1) * 128])
        fold[f'w{p}'] = bf(wmat)
        fold[f'diag{p}'] = bf(diag)
        fold[f'b{p}'] = bias
    common = {
        'wq': fold['wq'], 'wk': fold['wk'], 'wv': fold['wv'],
        'diagq': fold['diagq'], 'diagk': fold['diagk'], 'diagv': fold['diagv'],
        'bq': np.ascontiguousarray(fold['bq'].reshape(3, 128).T),
        'bk': np.ascontiguousarray(fold['bk'].reshape(3, 128).T),
        'bv': bf(fold['bv'].reshape(1, C)),
        'ok': bf(np.asarray(inputs['out_kernel'], dtype=np.float32).reshape(C, C)),
    }
    return common


def kernel(**inputs):
    if 'nc' not in _cache:
        _cache['nc'] = _build_nc()
    nc = _cache['nc']
    common = _fold_weights(inputs)
    bf = ml_dtypes.bfloat16
    xq = np.asarray(inputs['inputs_q'], dtype=np.float32).reshape(B, NTOK, C).astype(bf)
    xkv = np.asarray(inputs['inputs_kv'], dtype=np.float32).reshape(B, NTOK, C).astype(bf)
    in_maps = [dict(common, xq=np.ascontiguousarray(xq[b]),
                    xkv=np.ascontiguousarray(xkv[b])) for b in range(B)]
    res = run_bass_kernel_spmd(nc, in_maps, list(range(B)), trace=False)
    out = np.stack([res.results[b]['out'] for b in range(B)], axis=0)
    return out.astype(np.float32)
